# revision 1
# baseline (speedup 1.0000x reference)
"""Trainium2 Bass kernel for nn_MHParallelAttention (B=4,S=1024,H=16,DK=64).

Sharding: 8 cores = (batch) x (query-row half); each core owns output rows
[b, s0:s0+512, :] end-to-end, no collectives.

Algebra folds:
  * sum_h Wc_h*(q_h . k_h) == (concat_h Wc_h*q_h) . (concat_h k_h): the
    whole scores+head-combine collapses to one [512,1024]@[1024,1024]^T
    matmul per core, PSUM-accumulated over 8 chunks of 128 features.
  * bc is softmax-shift-invariant -> dropped.
  * block-diagonal [[W,0],[0,W]] 128x128 projection weights process a head
    PAIR per matmul with output at PSUM base partition 0 (fp32r-legal) and
    full 128-lane tanh.
  * softmax without max-subtraction (logits bounded ~6 for this problem);
    mask applied as 0/1 int8 multiply AFTER exp, fused with the row-sum in
    one DVE scalar_tensor_tensor op per half.

Schedule: input DMAs interleaved kt_j/qt_j in consumption order (engines
are in-order); scores for t=0,1 (both ki halves) accumulate inline with
the projections and ship their output rows early; t=2,3 follow with t=2
pre-running on spare PSUM banks. Matmuls run in float32r (1 cycle/row;
HW rel-err ~4e-4). Set KERNEL_F32R=0 for full fp32 (~2x slower).

Host-side prep is layout-only; all FLOPs run on device.
"""

import os
import sys

import numpy as np

for _p in ("/opt/trn_rl_repo", "/root/.axon_site/_ro/trn_rl_repo"):
    if os.path.isdir(_p) and _p not in sys.path:
        sys.path.insert(0, _p)

import concourse.bass as bass
import concourse.mybir as mybir
import concourse.tile as tile
from concourse import bacc
from concourse.bass import ds, ts

H, DK = 16, 64
B, S = 4, 1024
SQ = 512
NCORES = 8
NJ = 8
NEG = -1.0e10

F32 = mybir.dt.float32
F32R = mybir.dt.float32r
I32 = mybir.dt.int32
I8 = mybir.dt.int8

USE_F32R = os.environ.get("KERNEL_F32R", "1") == "1"

# packed weight layout along free dim: wkblk[128] | wqblk[128] | bk | bq | wc[8]
WOFF_WK, WOFF_WQ, WOFF_BK, WOFF_BQ, WOFF_WC = 0, 128, 256, 257, 258
WFREE = 266


def build_nc():
    nc = bacc.Bacc(None, target_bir_lowering=False, debug=False)
    DT = F32R if USE_F32R else F32

    qT = nc.dram_tensor("qT", [NJ, 128, SQ], DT, kind="ExternalInput")
    kT = nc.dram_tensor("kT", [NJ, 128, S], DT, kind="ExternalInput")
    msk = nc.dram_tensor("msk", [4, 128, S], I8, kind="ExternalInput")
    wts = nc.dram_tensor("wts", [128, WFREE], DT, kind="ExternalInput")
    out = nc.dram_tensor("out", [SQ, S], F32, kind="ExternalOutput")

    Tanh = mybir.ActivationFunctionType.Tanh
    Exp = mybir.ActivationFunctionType.Exp

    with tile.TileContext(nc) as tc:
        with (
            tc.tile_pool(name="const", bufs=1) as cst,
            tc.tile_pool(name="kin", bufs=1) as kin,
            tc.tile_pool(name="qin", bufs=1) as qin,
            tc.tile_pool(name="kpp", bufs=1) as kpp,
            tc.tile_pool(name="qpp", bufs=1) as qpp,
            tc.tile_pool(name="tmp", bufs=4) as tmpp,
            tc.tile_pool(name="mrow", bufs=1) as mrp,
            tc.tile_pool(name="soft", bufs=2) as softp,
            tc.tile_pool(name="stat", bufs=8) as statp,
            tc.tile_pool(name="obuf", bufs=4) as obp,
            tc.tile_pool(name="pproj", bufs=2, space="PSUM") as pproj,
            tc.tile_pool(name="pscore", bufs=4, space="PSUM") as pscore,
        ):
            wts_sb = cst.tile([128, WFREE], DT, tag="wts")
            nc.sync.dma_start(out=wts_sb[:], in_=wts[:])
            wkb = wts_sb[:, WOFF_WK:WOFF_WK + 128]
            wqb = wts_sb[:, WOFF_WQ:WOFF_WQ + 128]
            bkb = wts_sb[:, WOFF_BK:WOFF_BK + 1].bitcast(F32)
            bqb = wts_sb[:, WOFF_BQ:WOFF_BQ + 1].bitcast(F32)
            wcb = wts_sb[:, WOFF_WC:WOFF_WC + NJ].bitcast(F32)

            mk = mrp.tile([128, 4, S], I8, tag="mk")

            kp = [kpp.tile([128, S], DT, tag=f"kp{j}", name=f"kp{j}")
                  for j in range(NJ)]
            qp = [qpp.tile([128, SQ], DT, tag=f"qp{j}", name=f"qp{j}")
                  for j in range(NJ)]

            # ---- input DMAs on SP queue; arrival order = emission order =
            # consumption order. Fine granularity so ACT starts early.
            pst01 = {(t, kh): pscore.tile([128, 512], F32, tag="ps", bufs=6,
                     name=f"psA_{t}_{kh}") for t in range(2) for kh in range(2)}

            # kt_j then qt_j arrivals, each followed immediately by its
            # projection and the j-th kh=0 score chunk
            for j in range(NJ):
                kt = kin.tile([128, S], DT, tag="kt", bufs=4, name=f"kt{j}")
                nc.sync.dma_start(out=kt[:], in_=kT[j])
                qt = qin.tile([128, SQ], DT, tag="qt", bufs=4, name=f"qt{j}")
                nc.sync.dma_start(out=qt[:], in_=qT[j])
                for half in range(2):
                    ps = pproj.tile([128, 512], F32, tag="pp")
                    sl = ds(half * 512, 512)
                    nc.tensor.matmul(ps[:], wkb, kt[:, sl])
                    nc.scalar.activation(kp[j][:, sl], ps[:], Tanh, bias=bkb)
                ps = pproj.tile([128, 512], F32, tag="pp")
                nc.tensor.matmul(ps[:], wqb, qt[:])
                tq = tmpp.tile([128, SQ], F32, tag="tmp")
                nc.scalar.activation(tq[:], ps[:], Tanh, bias=bqb)
                nc.vector.tensor_scalar_mul(qp[j][:], tq[:], wcb[:, j:j + 1])
                for t in range(2):
                    for kh in range(2):
                        nc.tensor.matmul(
                            pst01[(t, kh)][:], qp[j][:, ts(t, 128)],
                            kp[j][:, ts(kh, 512)],
                            start=(j == 0), stop=(j == NJ - 1),
                        )

            # mask after inputs on the same queue (needed only by the tail)
            nc.sync.dma_start(out=mk[:], in_=msk[:].rearrange("t p k -> p t k"))

            # softmax without max-subtraction (|logit| <= ~6 here; masked
            # entries killed by multiplying with the 0/1 int8 mask AFTER exp;
            # fused accum gives the masked row-sum in the same DVE pass)
            exs = [softp.tile([128, S], F32, tag=f"ex{t}", name=f"ex{t}",
                              bufs=1) for t in range(4)]

            def tail_chain(t, psa, psb):
                nc.scalar.activation(exs[t][:, ts(0, 512)], psa[:], Exp)
                nc.scalar.activation(exs[t][:, ts(1, 512)], psb[:], Exp)
                exm = obp.tile([128, S], F32, tag="exm")
                s0 = statp.tile([128, 1], F32, tag="s0")
                s1 = statp.tile([128, 1], F32, tag="s1")
                nc.vector.scalar_tensor_tensor(
                    exm[:, ts(0, 512)], exs[t][:, ts(0, 512)], 1.0,
                    mk[:, t, ts(0, 512)],
                    op0=mybir.AluOpType.bypass, op1=mybir.AluOpType.mult,
                    accum_out=s0[:],
                )
                nc.vector.scalar_tensor_tensor(
                    exm[:, ts(1, 512)], exs[t][:, ts(1, 512)], 1.0,
                    mk[:, t, ts(1, 512)],
                    op0=mybir.AluOpType.bypass, op1=mybir.AluOpType.mult,
                    accum_out=s1[:],
                )
                ssum = statp.tile([128, 1], F32, tag="ssum")
                nc.vector.tensor_tensor(ssum[:], s0[:], s1[:],
                                        op=mybir.AluOpType.add)
                rec = statp.tile([128, 1], F32, tag="rec")
                nc.vector.reciprocal(rec[:], ssum[:])
                ot = obp.tile([128, S], F32, tag="ot")
                for hh in range(2):
                    nc.vector.tensor_scalar_mul(
                        ot[:, ts(hh, 512)], exm[:, ts(hh, 512)], rec[:])
                    nc.sync.dma_start(
                        out=out[ts(t, 128), ds(hh * 512, 512)],
                        in_=ot[:, ts(hh, 512)])

            # t=0,1 finished in phase 1 -> chain + output immediately
            for t in range(2):
                tail_chain(t, pst01[(t, 0)], pst01[(t, 1)])

            # ---- phase 2: t=2,3 (t=2 psums pre-run on spare banks)
            for t in (2, 3):
                psa = pscore.tile([128, 512], F32, tag="ps", bufs=6,
                                  name=f"psB_{t}_0")
                psb = pscore.tile([128, 512], F32, tag="ps", bufs=6,
                                  name=f"psB_{t}_1")
                for j in range(NJ):
                    nc.tensor.matmul(
                        psa[:], qp[j][:, ts(t, 128)], kp[j][:, ts(0, 512)],
                        start=(j == 0), stop=(j == NJ - 1),
                    )
                    nc.tensor.matmul(
                        psb[:], qp[j][:, ts(t, 128)], kp[j][:, ts(1, 512)],
                        start=(j == 0), stop=(j == NJ - 1),
                    )
                tail_chain(t, psa, psb)

    nc.compile()
    return nc


_NC = None


def _get_nc():
    global _NC
    if _NC is None:
        _NC = build_nc()
    return _NC


def make_in_maps(query, key, mask, Wq, bq, Wk, bk, Wc, bc):
    query = np.asarray(query, np.float32)
    key = np.asarray(key, np.float32)
    mask = np.asarray(mask)
    Wq = np.asarray(Wq, np.float32)
    Wk = np.asarray(Wk, np.float32)
    Wc = np.asarray(Wc, np.float32)
    bq = np.asarray(bq, np.float32)
    bk = np.asarray(bk, np.float32)

    def blockdiag(W):
        blk = np.zeros((128, 128), np.float32)
        blk[0:64, 0:64] = W.T
        blk[64:128, 64:128] = W.T
        return blk

    wts = np.zeros((128, WFREE), np.float32)
    wts[:, WOFF_WK:WOFF_WK + 128] = blockdiag(Wk)
    wts[:, WOFF_WQ:WOFF_WQ + 128] = blockdiag(Wq)
    wts[:, WOFF_BK] = np.tile(bk.reshape(-1), 2)
    wts[:, WOFF_BQ] = np.tile(bq.reshape(-1), 2)
    for j in range(NJ):
        wts[0:64, WOFF_WC + j] = Wc[0, 2 * j]
        wts[64:128, WOFF_WC + j] = Wc[0, 2 * j + 1]

    in_maps = []
    for c in range(NCORES):
        b, half = divmod(c, 2)
        s0 = half * SQ
        qh = query[b].reshape(H, S, DK)[:, s0:s0 + SQ, :]
        qTc = np.ascontiguousarray(qh.transpose(0, 2, 1)).reshape(NJ, 128, SQ)
        kh_ = key[b].reshape(H, S, DK)
        kTc = np.ascontiguousarray(kh_.transpose(0, 2, 1)).reshape(NJ, 128, S)
        mc = np.ascontiguousarray(
            mask[b, s0:s0 + SQ, :].reshape(4, 128, S)).astype(np.int8)
        in_maps.append({"qT": qTc, "kT": kTc, "msk": mc, "wts": wts})
    return in_maps


def kernel(query, key, mask, Wq, bq, Wk, bk, Wc, bc):
    from concourse.bass_utils import run_bass_kernel_spmd

    nc = _get_nc()
    in_maps = make_in_maps(query, key, mask, Wq, bq, Wk, bk, Wc, bc)
    res = run_bass_kernel_spmd(nc, in_maps, list(range(NCORES)))
    full = np.empty((B, S, S), np.float32)
    for c in range(NCORES):
        b, half = divmod(c, 2)
        full[b, half * SQ:(half + 1) * SQ, :] = res.results[c]["out"]
    return full



# revision 24
# speedup vs baseline: 1.2522x; 1.2522x over previous
"""Trainium2 Bass kernel for nn_MHParallelAttention (B=4,S=1024,H=16,DK=64).

Sharding: 8 cores = (batch) x (query-row half); each core owns output rows
[b, s0:s0+512, :] end-to-end, no collectives.

Algebra folds (as baseline):
  * sum_h Wc_h*(q_h . k_h) == (concat_h Wc_h*q_h) . (concat_h k_h): scores +
    head-combine collapse to one [512,1024]@[1024,1024]^T matmul per core,
    PSUM-accumulated over 8 chunks of 128 features.
  * bc is softmax-shift-invariant -> dropped.
  * block-diagonal [[W,0],[0,W]] 128x128 projection weights process a head
    PAIR per matmul.
  * softmax without max-subtraction (logits bounded ~7 here); 0/1 int8 mask
    multiplied AFTER exp, fused with the row-sum via scalar_tensor_tensor.

v2 changes vs baseline (39949ns -> ~31900ns predicted):
  * fp16 everywhere on the matmul paths (inputs, projections, exp, output):
    halves DMA bytes (8.5MB -> ~4.2MB/core); f16 matmuls run 1 col/cycle.
  * 2-bank [128,1024] PSUM tiles so tanh/exp run as single fat ACT
    instructions (amortizes the ~370-cycle ACT init overhead).
  * Q-first schedule: all 4 q-projection pairs run before the k loop, parking
    their psums in the slots that later hold ps0/ps1, so the k loop keeps
    t0 AND t1 score accumulators live (8 psum banks exactly).
  * scores lag the k loop by two j's so the ACT tanh stream never waits on
    PE (kproj_{j+1} is done when tanh-k_j retires).
  * phase B (t2,t3) uses per-half [128,512] psum tiles so each exp fires as
    soon as its own 8-matmul group stops (deps are tile-granular).
  * dual DGE paths: q chunks + weights on SP/HWDGE, k chunks split across
    Pool/SWDGE + SP so the per-DMA issue ladders overlap; ACT table load
    primed by a dummy tanh at t=0; chunked DMAs in consumption order.
  * fp16 output, upcast on host.
"""

import os
import sys

import numpy as np

for _p in ("/opt/trn_rl_repo", "/root/.axon_site/_ro/trn_rl_repo"):
    if os.path.isdir(_p) and _p not in sys.path:
        sys.path.insert(0, _p)

import concourse.bass as bass
import concourse.mybir as mybir
import concourse.tile as tile
from concourse import bacc
from concourse.bass import ds, ts

H, DK = 16, 64
B, S = 4, 1024
SQ = 512
NCORES = 8
NJ = 8

F16 = mybir.dt.float16
F32 = mybir.dt.float32
I8 = mybir.dt.int8


def build_nc():
    nc = bacc.Bacc(None, target_bir_lowering=False, debug=False)

    qT = nc.dram_tensor("qT", [NJ, 128, SQ], F16, kind="ExternalInput")
    kT = nc.dram_tensor("kT", [NJ, 128, S], F16, kind="ExternalInput")
    msk = nc.dram_tensor("msk", [4, 128, S], I8, kind="ExternalInput")
    w16 = nc.dram_tensor("w16", [128, 256], F16, kind="ExternalInput")
    w32 = nc.dram_tensor("w32", [128, 10], F32, kind="ExternalInput")
    out = nc.dram_tensor("out", [SQ, S], F16, kind="ExternalOutput")

    Tanh = mybir.ActivationFunctionType.Tanh
    Exp = mybir.ActivationFunctionType.Exp

    with tile.TileContext(nc) as tc:
        with (
            tc.tile_pool(name="const", bufs=1) as cst,
            tc.tile_pool(name="kin", bufs=1) as kin,
            tc.tile_pool(name="qin", bufs=1) as qin,
            tc.tile_pool(name="kpp", bufs=1) as kpp,
            tc.tile_pool(name="qpp", bufs=1) as qpp,
            tc.tile_pool(name="tqp", bufs=2) as tqp,
            tc.tile_pool(name="mrow", bufs=1) as mrp,
            tc.tile_pool(name="soft", bufs=2) as softp,
            tc.tile_pool(name="exmp", bufs=2) as exmp,
            tc.tile_pool(name="stat", bufs=8) as statp,
            tc.tile_pool(name="obuf", bufs=2) as obp,
            tc.tile_pool(name="pk", bufs=2, space="PSUM") as pkp,
            tc.tile_pool(name="pq", bufs=1, space="PSUM") as pqp,
            tc.tile_pool(name="sc0", bufs=1, space="PSUM") as sc0p,
            tc.tile_pool(name="sc1", bufs=1, space="PSUM") as sc1p,
        ):
            w16_sb = cst.tile([128, 256], F16, tag="w16")
            w32_sb = cst.tile([128, 10], F32, tag="w32")
            # prime the ACT table load (tanh/exp set) before real data lands
            prim = cst.tile([128, 1], F32, tag="prim")
            nc.vector.memset(prim[:], 0.0)
            nc.scalar.activation(prim[:], prim[:], Tanh)
            # two DGE paths in parallel: SP/HWDGE carries w16+k chunks,
            # Pool/SWDGE carries w32+q chunks+mask -> the per-DMA issue
            # ladders (~0.6-1.0us each) overlap instead of serializing
            nc.sync.dma_start(out=w16_sb[:], in_=w16[:])
            nc.sync.dma_start(out=w32_sb[:], in_=w32[:])
            wkb = w16_sb[:, ds(0, 128)]
            wqb = w16_sb[:, ds(128, 128)]
            bkb = w32_sb[:, ds(0, 1)]
            bqb = w32_sb[:, ds(1, 1)]
            wcb = w32_sb[:, ds(2, NJ)]

            # input chunks: per-j k tiles, alternating DGE paths
            kch = [
                kin.tile([128, 1, S], F16, tag=f"k{j}", name=f"k{j}")
                for j in range(NJ)
            ]
            qch = [
                qin.tile([128, 2, SQ], F16, tag=f"q{p}", name=f"q{p}")
                for p in range(4)
            ]
            def kt(j):
                return kch[j][:, 0, :]

            def qt(j):
                return qch[j // 2][:, j % 2, :]

            mk = mrp.tile([128, 4, S], I8, tag="mk")

            # DMA emission in consumption order (all on SP queue; transfers
            # serialize in order on the DMA engines)
            def dma_k(j, eng):
                eng.dma_start(out=kch[j][:],
                              in_=kT[ds(j, 1)].rearrange("j p k -> p j k"))

            def dma_q(p):
                nc.sync.dma_start(
                    out=qch[p][:],
                    in_=qT[ds(2 * p, 2)].rearrange("j p k -> p j k"))

            dma_q(0)
            dma_k(0, nc.gpsimd)
            dma_q(1)
            dma_k(1, nc.gpsimd)
            dma_q(2)
            dma_k(2, nc.gpsimd)
            dma_q(3)
            dma_k(3, nc.gpsimd)
            dma_k(4, nc.sync)
            dma_k(5, nc.gpsimd)
            dma_k(6, nc.sync)
            dma_k(7, nc.gpsimd)
            nc.sync.dma_start(out=mk[:],
                              in_=msk[:].rearrange("t p k -> p t k"))
            del dma_k, dma_q

            kp = [kpp.tile([128, S], F16, tag=f"kp{j}", name=f"kp{j}")
                  for j in range(NJ)]
            qp = [qpp.tile([128, SQ], F16, tag=f"qp{j}", name=f"qp{j}")
                  for j in range(NJ)]

            # ---- Q phase: all q projections first. The four [128,1024]
            # pair-psums use: sc0's slot, pq's slot, pk's two slots. After
            # their tanh-q completes those slots roll over to ps0/ps1/pk0/pk1,
            # so the k loop + both t0,t1 score accumulators fit in 8 banks.
            for p, pool, ptag in ((0, sc0p, "sc0"), (1, pqp, "pq"),
                                  (2, sc0p, "sc0"), (3, pqp, "pq")):
                pqt = pool.tile([128, S], F32, tag=ptag, name=f"pq{p}")
                nc.tensor.matmul(pqt[:, ds(0, 512)], wqb, qt(2 * p))
                nc.tensor.matmul(pqt[:, ds(512, 512)], wqb, qt(2 * p + 1))
                tq = tqp.tile([128, S], F16, tag="tq")
                nc.scalar.activation(tq[:], pqt[:], Tanh, bias=bqb)
                nc.vector.tensor_scalar_mul(
                    qp[2 * p][:], tq[:, ds(0, 512)], wcb[:, ds(2 * p, 1)])
                nc.vector.tensor_scalar_mul(
                    qp[2 * p + 1][:], tq[:, ds(512, 512)],
                    wcb[:, ds(2 * p + 1, 1)])

            # ---- K loop: kproj + tanh-k per j; scores for t0,t1 lag one j
            ps0 = sc0p.tile([128, S], F32, tag="sc0", name="ps0")
            ps1 = pqp.tile([128, S], F32, tag="pq", name="ps1")

            def sc_mm(pst, j, t, src512):
                nc.tensor.matmul(pst, qp[j][:, ts(t, 128)], src512,
                                 start=(j == 0), stop=(j == NJ - 1))

            def scores01(jj):
                sc_mm(ps0[:, ts(0, 512)], jj, 0, kp[jj][:, ts(0, 512)])
                sc_mm(ps0[:, ts(1, 512)], jj, 0, kp[jj][:, ts(1, 512)])
                sc_mm(ps1[:, ts(0, 512)], jj, 1, kp[jj][:, ts(0, 512)])
                sc_mm(ps1[:, ts(1, 512)], jj, 1, kp[jj][:, ts(1, 512)])

            for j in range(NJ):
                pk = pkp.tile([128, S], F32, tag="pk", name=f"pk{j}")
                nc.tensor.matmul(pk[:, ds(0, 512)], wkb, kt(j)[:, ds(0, 512)])
                nc.tensor.matmul(pk[:, ds(512, 512)], wkb,
                                 kt(j)[:, ds(512, 512)])
                # lag scores by 2 j's: kproj_{j+1} is already done when
                # tanh-k_j completes, so the ACT tanh stream never waits PE
                if j >= 2:
                    scores01(j - 2)
                nc.scalar.activation(kp[j][:], pk[:], Tanh, bias=bkb)
            scores01(6)
            scores01(7)

            exs = [softp.tile([128, S], F16, tag=f"ex{t}", name=f"ex{t}",
                              bufs=1) for t in range(4)]
            exm = [exmp.tile([128, S], F16, tag=f"exm{t}", name=f"exm{t}",
                             bufs=1) for t in range(4)]

            def exp_stt(t, pst, lo, w):
                sl = ds(lo, w)
                nc.scalar.activation(exs[t][:, sl], pst[:], Exp)
                ssum = statp.tile([128, 1], F32, tag="ssum",
                                  name=f"ssum{t}_{lo}")
                nc.vector.scalar_tensor_tensor(
                    exm[t][:, sl], exs[t][:, sl], 1.0, mk[:, t, sl],
                    op0=mybir.AluOpType.bypass, op1=mybir.AluOpType.mult,
                    accum_out=ssum[:],
                )
                return ssum

            def norm_out(t, sums):
                stot = sums[0]
                for i, s in enumerate(sums[1:]):
                    nxt = statp.tile([128, 1], F32, tag="stot",
                                     name=f"stot{t}_{i}")
                    nc.vector.tensor_tensor(nxt[:], stot[:], s[:],
                                            op=mybir.AluOpType.add)
                    stot = nxt
                rec = statp.tile([128, 1], F32, tag="rec", name=f"rec{t}")
                nc.vector.reciprocal(rec[:], stot[:])
                ot = obp.tile([128, S], F16, tag="ot")
                nc.vector.tensor_scalar_mul(ot[:], exm[t][:], rec[:])
                nc.sync.dma_start(out=out[ts(t, 128), :], in_=ot[:])

            # ---- phase B: t2, t3 in per-half [128,512] groups (tile-granular
            # deps let each exp fire right after its own group stops).
            # Slots: ps2a/ps2b <- pk pool (free after tanh-k_6/7), ps3a <-
            # sc0 (free after exp_t0), ps3b <- pq (free after exp_t1),
            # ps3c <- pk (free after exp_t2 kh0).
            s0 = exp_stt(0, ps0[:], 0, S)
            ps2a = pkp.tile([128, 512], F32, tag="pk", name="ps2a")
            for j in range(NJ):
                sc_mm(ps2a[:], j, 2, kp[j][:, ts(0, 512)])
            norm_out(0, [s0])
            s1 = exp_stt(1, ps1[:], 0, S)
            ps2b = pkp.tile([128, 512], F32, tag="pk", name="ps2b")
            for j in range(NJ):
                sc_mm(ps2b[:], j, 2, kp[j][:, ts(1, 512)])
            norm_out(1, [s1])
            s2a = exp_stt(2, ps2a[:], 0, 512)
            ps3a = sc0p.tile([128, 512], F32, tag="sc0", name="ps3a")
            for j in range(NJ):
                sc_mm(ps3a[:], j, 3, kp[j][:, ts(0, 512)])
            s2b = exp_stt(2, ps2b[:], 512, 512)
            norm_out(2, [s2a, s2b])
            ps3b = pqp.tile([128, 256], F32, tag="pq", name="ps3b")
            for j in range(NJ):
                sc_mm(ps3b[:], j, 3, kp[j][:, ds(512, 256)])
            s3a = exp_stt(3, ps3a[:], 0, 512)
            ps3c = pkp.tile([128, 256], F32, tag="pk", name="ps3c")
            for j in range(NJ):
                sc_mm(ps3c[:], j, 3, kp[j][:, ds(768, 256)])
            s3b = exp_stt(3, ps3b[:], 512, 256)
            s3c = exp_stt(3, ps3c[:], 768, 256)
            norm_out(3, [s3a, s3b, s3c])

    nc.compile()
    return nc


_NC = None


def _get_nc():
    global _NC
    if _NC is None:
        _NC = build_nc()
    return _NC


def make_in_maps(query, key, mask, Wq, bq, Wk, bk, Wc, bc):
    query = np.asarray(query, np.float32)
    key = np.asarray(key, np.float32)
    mask = np.asarray(mask)
    Wq = np.asarray(Wq, np.float32)
    Wk = np.asarray(Wk, np.float32)
    Wc = np.asarray(Wc, np.float32)
    bq = np.asarray(bq, np.float32)
    bk = np.asarray(bk, np.float32)

    def blockdiag(W):
        blk = np.zeros((128, 128), np.float32)
        blk[0:64, 0:64] = W.T
        blk[64:128, 64:128] = W.T
        return blk

    w16 = np.zeros((128, 256), np.float16)
    w16[:, 0:128] = blockdiag(Wk).astype(np.float16)
    w16[:, 128:256] = blockdiag(Wq).astype(np.float16)
    w32 = np.zeros((128, 10), np.float32)
    w32[:, 0] = np.tile(bk.reshape(-1), 2)
    w32[:, 1] = np.tile(bq.reshape(-1), 2)
    for j in range(NJ):
        w32[0:64, 2 + j] = Wc[0, 2 * j]
        w32[64:128, 2 + j] = Wc[0, 2 * j + 1]

    in_maps = []
    for c in range(NCORES):
        b, half = divmod(c, 2)
        s0 = half * SQ
        qh = query[b].reshape(H, S, DK)[:, s0:s0 + SQ, :]
        qTc = np.ascontiguousarray(
            qh.transpose(0, 2, 1)).reshape(NJ, 128, SQ).astype(np.float16)
        kh_ = key[b].reshape(H, S, DK)
        kTc = np.ascontiguousarray(
            kh_.transpose(0, 2, 1)).reshape(NJ, 128, S).astype(np.float16)
        mc = np.ascontiguousarray(
            mask[b, s0:s0 + SQ, :].reshape(4, 128, S)).astype(np.int8)
        in_maps.append({"qT": qTc, "kT": kTc, "msk": mc,
                        "w16": w16, "w32": w32})
    return in_maps


def kernel(query, key, mask, Wq, bq, Wk, bk, Wc, bc):
    from concourse.bass_utils import run_bass_kernel_spmd

    nc = _get_nc()
    in_maps = make_in_maps(query, key, mask, Wq, bq, Wk, bk, Wc, bc)
    res = run_bass_kernel_spmd(nc, in_maps, list(range(NCORES)))
    full = np.empty((B, S, S), np.float32)
    for c in range(NCORES):
        b, half = divmod(c, 2)
        full[b, half * SQ:(half + 1) * SQ, :] = \
            res.results[c]["out"].astype(np.float32)
    return full
